# revision 17
# baseline (speedup 1.0000x reference)
"""Trainium2 Bass kernel for nn_Convs4x44 (dense_cnn, memory-bound).

Pipeline per sample (64 input floats -> 4 output floats):
  conv1 2x2/s2 on 8x8 -> relu(x-0.2) -> conv2 2x2/s2 on 4x4 -> relu(x-2)
  -> 4->3 linear + relu -> 3->2 linear -> softmax(2)

Strategy: pure data parallel over 8 cores. Sample-major SBUF layout
[128 partitions, C samples/partition * 64 feats] so both elementwise engines
run with all 128 lanes busy and the HBM loads are big contiguous blocks.

Every weighted-sum chain is normalized by its leading weight so the chain is
pure DVE scalar_tensor_tensor accumulation (out = in1 + scalar*in0) and the
leading weight + bias fold for free into the ACT activation that follows
(out = func(scale*x + bias)). DVE therefore runs only the unavoidable
2-input accumulates; ACT runs one activation per chain. softmax(2) is
computed exactly as sigmoid(+/-(raw0-raw1)). GPSIMD is deliberately idle:
its elementwise ops measured 3-20x slower and its SBUF port contention
slowed concurrent DVE ops ~3x.
"""

import os

import numpy as np

import concourse.bass as bass
import concourse.tile as tile
from concourse import mybir
from concourse.bass_utils import run_bass_kernel_spmd


def _split_multiwaits(nc):
    """This container's walrus build supports only ONE sync-wait command per
    instruction ("Too many sync wait commands" otherwise), while Tile freely
    emits multi-wait instructions. Split every instruction with N>1 waits
    into (N-1) same-engine NoOps carrying one wait each, inserted before it
    in the basic block; per-engine execution order is block order filtered
    by engine, so semantics are unchanged."""
    for func in nc.m.functions:
        for blk in func.blocks:
            insts = blk.instructions
            out = []
            changed = False
            for ins in insts:
                si = ins.sync_info
                if si is not None and len(si.on_wait) > 1:
                    waits = list(si.on_wait)
                    for k, w in enumerate(waits[:-1]):
                        nop = mybir.InstNoOp(
                            name=f"{ins.name}-wsplit-{k}", ins=[], outs=[])
                        nop.engine = ins.engine
                        nop.sync_info = mybir.SyncInfo(on_wait=[w], on_update=[])
                        out.append(nop)
                    ins.sync_info = mybir.SyncInfo(
                        on_wait=[waits[-1]], on_update=list(si.on_update))
                    changed = True
                out.append(ins)
            if changed:
                insts[:] = out


N_CORES = 8
B = 1048576
BC = B // N_CORES          # samples per core
P = 128                    # SBUF partitions
# per-tile samples-per-partition; sum * P == BC
TILE_CS = [256, 256, 256, 256]
assert sum(TILE_CS) * P == BC

F32 = mybir.dt.float32
ALU = mybir.AluOpType
AF = mybir.ActivationFunctionType

# columns in the broadcast-constant tile (all chains normalized by their
# leading weight; `rX` denotes ratio weight/lead, `sc` the lead itself)
C1R = 0            # conv1: [k01/k00, k10/k00, k11/k00]
C1S = 3            # k00
C2R = 4            # conv2: [m01/m00, m10/m00, m11/m00]
C2S = 7            # m00
W1R = 8            # W1 ratios, row j at 8+3j: [W1j1/W1j0, W1j2/W1j0, W1j3/W1j0]
W1S = 17           # W1j0 for j=0..2
B1C = 20           # b1[j]
W2R = 23           # W2 ratios, row j at 23+2j: [W2j1/W2j0, W2j2/W2j0]
W2S = 27           # W2j0 for j=0..1
B2C = 29           # b2[j]
SH1 = 31           # -0.2 (conv1 relu shift)
SH2 = 32           # -2.0 (f relu shift)
NW = 33


def _build(reps=1):
    nc = bass.Bass("TRN2", target_bir_lowering=False, debug=False,
                   num_devices=N_CORES)
    x = nc.dram_tensor("x", [BC, 64], F32, kind="ExternalInput")
    wconst = nc.dram_tensor("wconst", [P, NW], F32, kind="ExternalInput")
    out = nc.dram_tensor("out", [BC, 4], F32, kind="ExternalOutput")

    with tile.TileContext(nc) as tc:
        with (
            tc.tile_pool(name="consts", bufs=1) as cpool,
            tc.tile_pool(name="x", bufs=2) as xpool,
            tc.tile_pool(name="mid", bufs=2) as mpool,
            tc.tile_pool(name="small", bufs=2) as spool,
            tc.tile_pool(name="out", bufs=2) as opool,
        ):
            ws = cpool.tile([P, NW], F32)
            nc.sync.dma_start(ws[:], wconst.ap()[:])

            def sc(col):
                return ws[:, col:col + 1]

            CMAX = max(TILE_CS)
            s0 = 0
            for ti, C in enumerate(
                    [c for _ in range(reps) for c in TILE_CS]):
                if s0 >= BC:
                    s0 = 0
                # big loads on the idle SP ring; small stores on the ACT
                # ring (sharing the SP ring with loads cost ~9 us/pass,
                # loads on the ACT ring serialize behind activations)
                ldq = nc.sync
                stq = nc.scalar
                ns = P * C
                x_view = x.ap()[s0:s0 + ns, :].rearrange(
                    "(p c) f -> p (c f)", p=P, c=C)
                out_view = out.ap()[s0:s0 + ns, :].rearrange(
                    "(p c) four -> p (c four)", p=P, c=C)
                s0 += ns

                xt = xpool.tile([P, CMAX * 64], F32, tag="xt")
                ldq.dma_start(xt[:, :C * 64], x_view)

                # conv1 normalized by k00, serial in-place chain:
                # t1 = A + r0*B + r1*C + r2*D, taps A=(0,0) B=(0,1) C=(1,0)
                # D=(1,1); x1 = relu(k00*t1 - 0.2)
                xv = xt[:, :C * 64].rearrange(
                    "p (c oh ti ow tj) -> p c oh ti ow tj", oh=4, ti=2, ow=4,
                    tj=2)
                t1 = mpool.tile([P, CMAX * 16], F32, tag="t1")
                t1v = t1[:, :C * 16].rearrange("p (c oh ow) -> p c oh ow",
                                               oh=4, ow=4)
                nc.vector.scalar_tensor_tensor(
                    t1v, xv[:, :, :, 0, :, 1], sc(C1R + 0),
                    xv[:, :, :, 0, :, 0], ALU.mult, ALU.add)
                nc.vector.scalar_tensor_tensor(
                    t1v, xv[:, :, :, 1, :, 0], sc(C1R + 1), t1v,
                    ALU.mult, ALU.add)
                nc.vector.scalar_tensor_tensor(
                    t1v, xv[:, :, :, 1, :, 1], sc(C1R + 2), t1v,
                    ALU.mult, ALU.add)
                # in-place shifted relu folds the k00 scale
                nc.scalar.activation(t1[:, :C * 16], t1[:, :C * 16], AF.Relu,
                                     bias=sc(SH1), scale=sc(C1S))

                # conv2 on the 4x4 maps, same normalized serial form
                x1v = t1[:, :C * 16].rearrange(
                    "p (c oh ti ow tj) -> p c oh ti ow tj", oh=2, ti=2, ow=2,
                    tj=2)
                t2 = spool.tile([P, CMAX * 4], F32, tag="t2")
                t2v = t2[:, :C * 4].rearrange("p (c oh ow) -> p c oh ow",
                                              oh=2, ow=2)
                nc.vector.scalar_tensor_tensor(
                    t2v, x1v[:, :, :, 0, :, 1], sc(C2R + 0),
                    x1v[:, :, :, 0, :, 0], ALU.mult, ALU.add)
                nc.vector.scalar_tensor_tensor(
                    t2v, x1v[:, :, :, 1, :, 0], sc(C2R + 1), t2v,
                    ALU.mult, ALU.add)
                nc.vector.scalar_tensor_tensor(
                    t2v, x1v[:, :, :, 1, :, 1], sc(C2R + 2), t2v,
                    ALU.mult, ALU.add)
                # f = relu(m00*t2 - 2), in place
                nc.scalar.activation(t2[:, :C * 4], t2[:, :C * 4], AF.Relu,
                                     bias=sc(SH2), scale=sc(C2S))
                fv = t2[:, :C * 4].rearrange("p (c i) -> p c i", i=4)

                # h_j = relu(W1j0*(f0 + r1 f1 + r2 f2 + r3 f3) + b1_j),
                # j-major [3C], relu in place
                h = spool.tile([P, CMAX * 3], F32, tag="h")
                for j in range(3):
                    hj = h[:, j * C:(j + 1) * C]
                    nc.vector.scalar_tensor_tensor(
                        hj, fv[:, :, 1], sc(W1R + 3 * j), fv[:, :, 0],
                        ALU.mult, ALU.add)
                    for i in (2, 3):
                        nc.vector.scalar_tensor_tensor(
                            hj, fv[:, :, i], sc(W1R + 3 * j + i - 1), hj,
                            ALU.mult, ALU.add)
                    nc.scalar.activation(hj, hj, AF.Relu,
                                         bias=sc(B1C + j), scale=sc(W1S + j))
                hrv = h[:, :C * 3].rearrange("p (j c) -> p j c", j=3)

                # raw_j = W2j0*(h0 + s1 h1 + s2 h2) + b2_j
                # out tile layout per sample: [cls0, cls1, raw0, raw1]
                ot = opool.tile([P, CMAX * 4], F32, tag="ot")
                ov = ot[:, :C * 4].rearrange("p (c four) -> p c four", four=4)
                v = spool.tile([P, CMAX * 2], F32, tag="v")
                for j in range(2):
                    vj = v[:, j * C:(j + 1) * C]
                    nc.vector.scalar_tensor_tensor(
                        vj, hrv[:, 1, :], sc(W2R + 2 * j), hrv[:, 0, :],
                        ALU.mult, ALU.add)
                    nc.vector.scalar_tensor_tensor(
                        vj, hrv[:, 2, :], sc(W2R + 2 * j + 1), vj,
                        ALU.mult, ALU.add)
                    nc.scalar.activation(ov[:, :, 2 + j], vj, AF.Identity,
                                         bias=sc(B2C + j), scale=sc(W2S + j))

                # softmax over 2 classes: cls0 = sigmoid(raw0-raw1)
                d = spool.tile([P, CMAX], F32, tag="d")
                nc.vector.tensor_sub(d[:, :C], ov[:, :, 2], ov[:, :, 3])
                nc.scalar.activation(ov[:, :, 0], d[:, :C], AF.Sigmoid,
                                     bias=0.0, scale=1.0)
                nc.scalar.activation(ov[:, :, 1], d[:, :C], AF.Sigmoid,
                                     bias=0.0, scale=-1.0)

                stq.dma_start(out_view, ot[:, :C * 4])

    _split_multiwaits(nc)
    return nc


_NC = None


def _get_nc():
    global _NC
    if _NC is None:
        _NC = _build()
    return _NC


def _wconst_row(conv1_w, conv2_w, W1, b1, W2, b2):
    k = np.asarray(conv1_w, dtype=np.float32).reshape(4)
    m = np.asarray(conv2_w, dtype=np.float32).reshape(4)
    W1 = np.asarray(W1, dtype=np.float32).reshape(3, 4)
    b1 = np.asarray(b1, dtype=np.float32).reshape(3)
    W2 = np.asarray(W2, dtype=np.float32).reshape(2, 3)
    b2 = np.asarray(b2, dtype=np.float32).reshape(2)
    row = np.zeros(NW, dtype=np.float64)
    row[C1R:C1R + 3] = [k[1] / k[0], k[2] / k[0], k[3] / k[0]]
    row[C1S] = k[0]
    row[C2R:C2R + 3] = [m[1] / m[0], m[2] / m[0], m[3] / m[0]]
    row[C2S] = m[0]
    for j in range(3):
        row[W1R + 3 * j:W1R + 3 * j + 3] = W1[j, 1:] / W1[j, 0]
        row[W1S + j] = W1[j, 0]
        row[B1C + j] = b1[j]
    for j in range(2):
        row[W2R + 2 * j:W2R + 2 * j + 2] = W2[j, 1:] / W2[j, 0]
        row[W2S + j] = W2[j, 0]
        row[B2C + j] = b2[j]
    row[SH1] = -0.2
    row[SH2] = -2.0
    return row.astype(np.float32)


TIMED_REPS = 32


def _timed(np_inputs, iters=16, reps=TIMED_REPS):
    """Measure steady-state per-pass HW time.

    Builds a timing variant of the kernel that repeats the full pipeline
    `reps` times inside one NEFF execution (re-reading the same HBM input),
    so device time per call (~reps * pass) dwarfs host dispatch (~1 ms) and
    the axon sync overhead (~75 ms) cancels in a two-burst slope. Calls are
    serialized by donation-chaining the output buffer.
    """
    import time

    import jax
    import jax.core
    import jax.numpy as jnp
    from jax.experimental.shard_map import shard_map
    from jax.sharding import Mesh, NamedSharding, PartitionSpec

    from concourse import bass2jax as b2j

    x = np.ascontiguousarray(
        np.asarray(np_inputs["x"], dtype=np.float32).reshape(B, 64))
    row = _wconst_row(np_inputs["conv1_w"], np_inputs["conv2_w"],
                      np_inputs["W1"], np_inputs["b1"], np_inputs["W2"],
                      np_inputs["b2"])
    wconst = np.ascontiguousarray(
        np.tile(row[None, :], (P * N_CORES, 1)).astype(np.float32))

    nc = _build(reps=reps)
    b2j.install_neuronx_cc_hook()
    devices = jax.devices()[:N_CORES]
    mesh = Mesh(np.asarray(devices), ("core",))
    spec = PartitionSpec("core")
    sh = NamedSharding(mesh, spec)
    out_aval = jax.core.ShapedArray((BC, 4), jnp.float32)

    def _body(xs, ws, zs):
        outs = b2j._bass_exec_p.bind(
            xs, ws, zs, b2j.partition_id_tensor(),
            out_avals=(out_aval,),
            in_names=("x", "wconst", "out", "partition_id"),
            out_names=("out",),
            lowering_input_output_aliases=(),
            sim_require_finite=True,
            sim_require_nnan=True,
            nc=nc,
        )
        return outs[0]

    fn = jax.jit(
        shard_map(_body, mesh=mesh, in_specs=(spec, spec, spec),
                  out_specs=spec, check_rep=False),
        donate_argnums=(2,), keep_unused=True)

    X = jax.device_put(x, sh)
    W = jax.device_put(wconst, sh)
    X.block_until_ready()
    W.block_until_ready()

    z = fn(X, W, np.zeros((B, 4), np.float32))
    z.block_until_ready()  # compile + warm

    def run_n(n, z):
        t0 = time.perf_counter()
        for _ in range(n):
            z = fn(X, W, z)
        z.block_until_ready()
        return time.perf_counter() - t0, z

    base = 2
    slopes = []
    for _ in range(5):
        t1, z = run_n(base, z)
        t2, z = run_n(base + iters, z)
        slopes.append((t2 - t1) / iters)
    slopes.sort()
    if os.environ.get("TIMED_VERBOSE"):
        print("slopes/pass us:",
              [f"{s / reps * 1e6:.1f}" for s in slopes], flush=True)
    ns = slopes[len(slopes) // 2] / reps * 1e9
    return ns, np.asarray(z)


def kernel(x, conv1_w, conv2_w, W1, b1, W2, b2):
    x = np.ascontiguousarray(np.asarray(x, dtype=np.float32)).reshape(B, 64)
    row = _wconst_row(conv1_w, conv2_w, W1, b1, W2, b2)
    wconst = np.ascontiguousarray(np.tile(row[None, :], (P, 1)))

    nc = _get_nc()
    in_maps = [
        {"x": np.ascontiguousarray(x[i * BC:(i + 1) * BC]), "wconst": wconst}
        for i in range(N_CORES)
    ]
    res = run_bass_kernel_spmd(nc, in_maps, core_ids=list(range(N_CORES)))
    out = np.concatenate([res.results[i]["out"] for i in range(N_CORES)], axis=0)
    classification = np.ascontiguousarray(out[:, 0:2])
    raw = np.ascontiguousarray(out[:, 2:4])
    return classification, raw
